# revision 6
# baseline (speedup 1.0000x reference)
"""Trainium2 Bass kernel for batched multi-head attention with per-batch mask.

Reference semantics (B=8, H=16, S=1024, D=64):
    scores = Q @ K^T                      # [B,H,S,S]
    scores = where(mask[b], -1e9, scores) # per-batch mask
    attn   = softmax(scores / sqrt(D))
    out    = attn @ V

Key observations used here:
  * A masked batch (mask[b]=True) has constant scores, so softmax is exactly
    uniform (1/S) and out[b,h,q,:] = mean_k V[b,h,k,:].  That degenerate case
    is computed directly on host; only unmasked (b,h) units go to the device.
  * For unmasked units |scores/8| <= ~7, so softmax without max-subtraction
    (exp(x)/sum exp(x)) is numerically safe and mathematically identical.
  * Unmasked units are embarrassingly parallel; they are balanced across the
    8 NeuronCores (H=16 and B=8 make the unit count divisible by 8).

Device algorithm per unit (S=1024 split into 8 chunks of 128 along k):
  mm1:  T[k,q]  = sum_d K[k,d]*Q[q,d]       (lhsT = K^T chunk, rhs = Q^T)
  exp:  E[k,q]  = exp(T/8)                  (ScalarE, PSUM->SBUF)
  mm2:  U[m,q] += sum_k Vx[k,m]*E[k,q]      (lhsT = Vx chunk = [V | ones])
        -> U[0:64,q] = unnormalized out^T, U[64,q] = softmax denominator
  out:  transpose U per 128-column block on PE, out = U^T[:, :64] * (1/denom)
"""

import numpy as np

B, H, S, D = 8, 16, 1024, 64
P = 128                      # SBUF partitions / k-chunk size
NCHUNK = S // P              # 8 k-chunks per unit
NQ = S // P                  # 8 q-blocks for the output transpose
NHALF = 2                    # matmul moving operand is limited to N=512 fp32
NCORES = 8

_program_cache = {}


def _build_program(n_units):
    import concourse.mybir as mybir
    import concourse.tile as tile
    from concourse import bacc

    f32 = mybir.dt.float32
    f32r = mybir.dt.float32r
    nc = bacc.Bacc("TRN2", target_bir_lowering=False, debug=False)

    qt_d = nc.dram_tensor("qt", [n_units, D, S], f32r, kind="ExternalInput").ap()
    kt_d = nc.dram_tensor("kt", [n_units, D, S], f32r, kind="ExternalInput").ap()
    vx_d = nc.dram_tensor("vx", [n_units, S, D + 1], f32r, kind="ExternalInput").ap()
    eye_d = nc.dram_tensor("eye", [P, P], f32, kind="ExternalInput").ap()
    out_d = nc.dram_tensor("out", [n_units, S, D], f32, kind="ExternalOutput").ap()

    with tile.TileContext(nc) as tc:
        with (
            tc.tile_pool(name="qp", bufs=2) as qp,
            tc.tile_pool(name="kp", bufs=2) as kp,
            tc.tile_pool(name="vp", bufs=2) as vp,
            tc.tile_pool(name="ep", bufs=3) as ep,
            tc.tile_pool(name="up", bufs=2) as up,
            tc.tile_pool(name="op", bufs=4) as op,
            tc.tile_pool(name="rp", bufs=4) as rp,
            tc.tile_pool(name="ip", bufs=1) as ip,
            tc.tile_pool(name="pt", bufs=2, space="PSUM") as pt,   # 2 banks each
            tc.tile_pool(name="pu", bufs=1, space="PSUM") as pu,   # 2 banks
            tc.tile_pool(name="po", bufs=2, space="PSUM") as po,   # 1 bank each
        ):
            eye = ip.tile([P, P], f32)
            nc.sync.dma_start(eye, eye_d)

            for j in range(n_units):
                qt = qp.tile([D, S], f32r)
                nc.sync.dma_start(qt, qt_d[j])
                kt = kp.tile([D, S], f32r)
                nc.sync.dma_start(kt, kt_d[j])
                vx = vp.tile([P, NCHUNK, D + 1], f32r)
                nc.sync.dma_start(vx, vx_d[j].rearrange("(c p) d -> p c d", p=P))

                u_ps = pu.tile([D + 1, S], f32)
                for c in range(NCHUNK):
                    t_ps = pt.tile([P, S], f32)
                    for h in range(NHALF):
                        qs = slice(h * 512, (h + 1) * 512)
                        nc.tensor.matmul(
                            t_ps[:, qs],
                            lhsT=kt[:, c * P:(c + 1) * P],
                            rhs=qt[:, qs],
                            start=True,
                            stop=True,
                        )
                    e_sb = ep.tile([P, S], f32r)
                    nc.scalar.activation(
                        e_sb, t_ps, mybir.ActivationFunctionType.Exp,
                        bias=0.0, scale=0.125,
                    )
                    for h in range(NHALF):
                        qs = slice(h * 512, (h + 1) * 512)
                        nc.tensor.matmul(
                            u_ps[:, qs],
                            lhsT=vx[:, c, :],
                            rhs=e_sb[:, qs],
                            start=(c == 0),
                            stop=(c == NCHUNK - 1),
                        )

                u_sb = up.tile([D + 1, S], f32)
                nc.vector.tensor_copy(out=u_sb, in_=u_ps)
                for qc in range(NQ):
                    o_ps = po.tile([P, D + 1], f32)
                    nc.tensor.transpose(
                        o_ps, u_sb[:, qc * P:(qc + 1) * P], eye[:D + 1, :D + 1]
                    )
                    r_sb = rp.tile([P, 1], f32)
                    nc.vector.reciprocal(out=r_sb, in_=o_ps[:, D:D + 1])
                    o_sb = op.tile([P, D], f32)
                    nc.vector.tensor_scalar_mul(
                        out=o_sb, in0=o_ps[:, 0:D], scalar1=r_sb
                    )
                    nc.sync.dma_start(out_d[j][qc * P:(qc + 1) * P, :], o_sb)
    nc.compile()
    return nc


def _get_program(n_units):
    if n_units not in _program_cache:
        _program_cache[n_units] = _build_program(n_units)
    return _program_cache[n_units]


def _round_fp32r(x):
    """Round fp32 to the fp32r-representable set (bf16 hi + bf16 lo pair).

    The walrus verifier requires fp32r matmul operands to be pre-rounded;
    the PE's replicated fp32 path decomposes each value into two bf16s.
    """
    import ml_dtypes

    hi = x.astype(ml_dtypes.bfloat16).astype(np.float32)
    lo = (x - hi).astype(ml_dtypes.bfloat16).astype(np.float32)
    return hi + lo


def _prepare(Q, K, V, mask):
    """Host-side sharding. Returns (out_skeleton, units_per_core, in_maps)."""
    Q = np.ascontiguousarray(Q, dtype=np.float32)
    K = np.ascontiguousarray(K, dtype=np.float32)
    V = np.ascontiguousarray(V, dtype=np.float32)
    mask_b = np.asarray(mask).reshape(B).astype(bool)

    out = np.empty((B, H, S, D), dtype=np.float32)

    # Masked batches: softmax over a constant row is exactly uniform -> mean of V.
    for b in np.nonzero(mask_b)[0]:
        mv = V[b].mean(axis=1, dtype=np.float32)          # [H, D]
        out[b] = np.broadcast_to(mv[:, None, :], (H, S, D))

    units = [(b, h) for b in range(B) if not mask_b[b] for h in range(H)]
    if not units:
        return out, None, None

    # Pad to a multiple of NCORES with duplicates (identical redundant work).
    n_per = -(-len(units) // NCORES)
    padded = units + [units[0]] * (n_per * NCORES - len(units))
    per_core = [padded[i::NCORES] for i in range(NCORES)]

    QT = Q.transpose(0, 1, 3, 2)                          # [B,H,D,S] views
    KT = K.transpose(0, 1, 3, 2)
    ones = np.ones((S, 1), dtype=np.float32)
    eye = np.eye(P, dtype=np.float32)

    in_maps = []
    for core_units in per_core:
        qt = _round_fp32r(
            np.ascontiguousarray(np.stack([QT[b, h] for b, h in core_units]))
        )
        kt = _round_fp32r(
            np.ascontiguousarray(np.stack([KT[b, h] for b, h in core_units]))
        )
        vx = _round_fp32r(
            np.ascontiguousarray(
                np.stack(
                    [np.concatenate([V[b, h], ones], axis=1) for b, h in core_units]
                )
            )
        )
        in_maps.append({"qt": qt, "kt": kt, "vx": vx, "eye": eye})
    return out, per_core, in_maps


def _run_device(n_units, in_maps, trace=False, trace_cores=None):
    from concourse import bass_utils

    nc = _get_program(n_units)
    return bass_utils.run_bass_kernel_spmd(
        nc,
        in_maps,
        list(range(NCORES)),
        trace=trace,
        trace_cores=trace_cores,
    )


def kernel(Q, K, V, mask, _trace=False, _result_box=None):
    out, per_core, in_maps = _prepare(Q, K, V, mask)
    if in_maps is None:
        return out
    res = _run_device(len(per_core[0]), in_maps, trace=_trace)
    if _result_box is not None:
        _result_box.append(res)
    for i, core_units in enumerate(per_core):
        core_out = res.results[i]["out"]
        for s, (b, h) in enumerate(core_units):
            out[b, h] = core_out[s]
    return out
